# revision 24
# baseline (speedup 1.0000x reference)
"""Trainium2 Bass kernel for nn_Attention_83330955478086.

Full attention layer: QKV projections + (degenerate) rotary + causal softmax
attention + output projection.  x:(1,2048,4096), 32 heads x 128 head_dim.

Sharding: tensor-parallel over heads. Each of the 8 cores computes 4 heads
(d-shard of 512) of Q/K/V, runs attention for those heads, then the cores
AllGather the attention outputs (one collective per 512-q chunk, pipelined
against the remaining attention work) and each computes a 512-column slice
of the final output projection.  Host concatenates the slices.

All matmul operands are bf16 (fp32 PSUM accumulation): full PE rate, fast
weight loads, half the HBM/collective bytes.  rel err ~6e-3 vs 2e-2 gate.

Key performance structure (measured 981us baseline -> ~750us):
  - fused Q+K projections: one sweep over x^T (512-seq chunks, 4+4 PSUM
    banks) emits Q^T and K^T [head_dim, seq] into resident SBUF
  - V pass: x tiles stationary, natural [seq,d] layout, resident in SBUF
  - attention per 512-q chunk: scoresT[k,q] matmuls; exp on ACT with the
    softmax 1/sqrt(HD) folded into the activation scale; the causal mask
    is applied multiplicatively AFTER the exp using resident deduped
    exp(mask) tiles (bf16 mul at full DVE rate); softmax denominators are
    accumulated on DVE in bf16 and reduced by ONE [128,128]-ones matmul
    per (head, chunk) whose output is the k-sum already broadcast across
    all partitions (no gpsimd partition_broadcast, no DVE-queue blocking)
  - one AllGather per 512-q chunk, fired as soon as that chunk's heads
    are done; the collectives serialize on the CC core (~30-40us each) so
    early firing keeps every gather off the critical path
  - p3 for qc0 rides inside qc3's attention window on 2 PSUM banks (the
    PE fills exp-wait bubbles with output-projection matmuls); qc1/qc2
    run as a stationary-paired pass, qc3 last
  - DMA queue discipline: x tiles head the sync queue at t0; wq/wk chunks
    beyond the first pair are paced by explicit deps on projection
    matmuls so the weight burst cannot starve the x stream; wv/wo follow
    on scalar (wo re-uses wq's SBUF, WAR-delayed past the QK pass); p3
    at_t loads are explicitly ordered after attention DMAs so the
    scheduler cannot hoist them ahead of the attention pipeline and stall
    it on an un-issued collective.

Rotary degenerates to an elementwise scale (the reference's pair-swap is the
identity): out[2j] = q[2j]*(c_j - s_j), out[2j+1] = q[2j+1]*(c_j + s_j).
We permute the wq/wk columns per head (even hd first, odd hd second, on the
host) so the device multiplies by a host-computed [cos-sin; cos+sin] stacked
tile without interleaved-partition access.  The permutation cancels in the
q.k contraction.
"""
import math
import os

import ml_dtypes
import numpy as np

import concourse.bacc as bacc
import concourse.tile as tile
from concourse.tile import add_dep_helper
from concourse import mybir
from concourse.bass_utils import run_bass_kernel_spmd

N_CORES = 8
S = 2048
D = 4096
H = 32
HD = 128
DSH = D // N_CORES  # 512 per-core d shard
HL = DSH // HD  # 4 heads per core
KT = D // 128  # 32 contraction tiles for the projections
SC = S // 512  # 4 seq chunks of 512
ST = S // 128  # 16 seq tiles of 128

F32 = mybir.dt.float32
BF16 = mybir.dt.bfloat16
BF16_NP = ml_dtypes.bfloat16

# mask-block classes (cls_grid entries: B_SKIP, B_ZERO, or pattern index >= 0)
B_SKIP = -1  # fully masked: exp(mask*scale) underflows to exactly 0 -> skip
B_ZERO = -2  # mask identically 0: skip the multiply
MAX_UNIQ = 16  # resident unique mask blocks (causal mask has 4)


def _build(cls_grid, n_uniq):
    nc = bacc.Bacc(
        "TRN2", target_bir_lowering=False, debug=False, num_devices=N_CORES
    )

    xT = nc.dram_tensor("xT", [D, S], BF16, kind="ExternalInput")
    wqT = nc.dram_tensor("wqT", [D, DSH], BF16, kind="ExternalInput")
    wkT = nc.dram_tensor("wkT", [D, DSH], BF16, kind="ExternalInput")
    wvT = nc.dram_tensor("wvT", [D, DSH], BF16, kind="ExternalInput")
    woT = nc.dram_tensor("woT", [D, DSH], BF16, kind="ExternalInput")
    gkT = nc.dram_tensor("gkT", [128, S], F32, kind="ExternalInput")
    ones_in = nc.dram_tensor("ones_in", [128, 128], BF16, kind="ExternalInput")
    nu = max(n_uniq, 1)
    emu_in = nc.dram_tensor("emu_in", [128, nu * 512], BF16, kind="ExternalInput")
    outT = nc.dram_tensor("outT", [DSH, S], F32, kind="ExternalOutput")

    attn_sc = [
        nc.dram_tensor(f"attn_sc{i}", [DSH, 512], BF16) for i in range(SC)
    ]
    attn_full = [
        nc.dram_tensor(f"attn_full{i}", [D, 512], BF16, addr_space="Shared")
        for i in range(SC)
    ]

    def w_chunk(w_sb, dram, g, n_chunks, eng, dep=None):
        """Load weight chunk g of n_chunks; optionally pace via dep on an
        earlier matmul so the burst cannot starve the x stream."""
        kt_per = KT // n_chunks
        ld = eng.dma_start(
            w_sb[:, g * kt_per * DSH : (g + 1) * kt_per * DSH].rearrange(
                "p (t d) -> p t d", d=DSH
            ),
            dram.ap()[
                g * kt_per * 128 : (g + 1) * kt_per * 128, :
            ].rearrange("(t p) d -> p t d", p=128),
        )
        if dep is not None:
            add_dep_helper(ld.ins, dep.ins, reason="paced weight prefetch")
        return ld

    with tile.TileContext(nc) as tc, tc.tile_pool(
        name="persist", bufs=1
    ) as pp:
        # ---- long-lived SBUF tensors ----
        wq_sb = pp.tile([128, KT * DSH], BF16, name="wq_sb")
        wk_sb = pp.tile([128, KT * DSH], BF16, name="wk_sb")
        wv_sb = pp.tile([128, KT * DSH], BF16, name="wv_sb")
        q_sb = pp.tile([128, HL * S], BF16, name="q_sb")  # head-major Q^T
        k_sb = pp.tile([128, HL * S], BF16, name="k_sb")  # head-major K^T
        v_sb = pp.tile([128, ST * DSH], BF16, name="v_sb")  # s-tile-major V
        gk = pp.tile([128, S], F32, name="gk")  # rotary multipliers
        ones_t = pp.tile([128, 128], BF16, name="ones_t")
        emu = pp.tile([128, nu * 512], BF16, name="emu")

        # first chunks upfront (covers kc 0..7), fine-grained and on two
        # queues so the first kc's wq/wk pair lands as early as possible
        for g in range(4):
            w_chunk(wq_sb, wqT, g, 16, nc.gpsimd)
            w_chunk(wk_sb, wkT, g, 16, nc.scalar)
        # small constants on scalar (sync is reserved for the x stream)
        nc.scalar.dma_start(gk[:], gkT.ap())
        nc.scalar.dma_start(ones_t[:], ones_in.ap())
        nc.scalar.dma_start(emu[:], emu_in.ap())

        # ---- phase 1a: fused Q^T + K^T projections (one sweep over x) ----
        with (
            tc.tile_pool(name="pxqk", bufs=4) as pxqk,
            tc.tile_pool(name="qk_ps", bufs=1, space="PSUM") as ps_qk,
        ):
            for sc in range(SC):  # 512-seq chunks
                psq = [
                    ps_qk.tile([128, 512], F32, name=f"psq{i}") for i in range(HL)
                ]
                psk = [
                    ps_qk.tile([128, 512], F32, name=f"psk{i}") for i in range(HL)
                ]
                for kc in range(KT):
                    xt = pxqk.tile([128, 512], BF16, name="xqk")
                    nc.sync.dma_start(
                        xt[:],
                        xT.ap()[
                            kc * 128 : (kc + 1) * 128, sc * 512 : (sc + 1) * 512
                        ],
                    )
                    qmm = None
                    for w_sb, psd in ((wq_sb, psq), (wk_sb, psk)):
                        for dt in range(HL):
                            qmm = nc.tensor.matmul(
                                psd[dt][:],
                                w_sb[
                                    :,
                                    kc * DSH + dt * 128 : kc * DSH + (dt + 1) * 128,
                                ],
                                xt[:],
                                start=(kc == 0),
                                stop=(kc == KT - 1),
                            )
                    if sc == 0:
                        # paced prefetches, anchored to this kc's matmul
                        if kc % 4 == 2 and 2 + kc // 4 < 8:
                            g = 2 + kc // 4
                            w_chunk(wq_sb, wqT, g, 8, nc.gpsimd, qmm)
                            w_chunk(wk_sb, wkT, g, 8, nc.gpsimd, qmm)
                    if sc == 1 and kc % 8 == 0:
                        w_chunk(wv_sb, wvT, kc // 8, 4, nc.scalar, qmm)
                for dt in range(HL):
                    qsl = q_sb[:, dt * S + sc * 512 : dt * S + (sc + 1) * 512]
                    if sc == SC - 1:
                        # last chunk: evacuate on ACT (copy) so the V pass
                        # can claim the PSUM banks ~2x sooner; the rotary
                        # multiply is applied in place during the V pass
                        nc.scalar.copy(qsl, psq[dt][:])
                    else:
                        nc.vector.tensor_mul(
                            qsl, psq[dt][:], gk[:, sc * 512 : (sc + 1) * 512]
                        )
                for dt in range(HL):
                    nc.vector.tensor_mul(
                        k_sb[:, dt * S + sc * 512 : dt * S + (sc + 1) * 512],
                        psk[dt][:],
                        gk[:, sc * 512 : (sc + 1) * 512],
                    )

        # deferred rotary multiplies for the last seq chunk (DVE is idle
        # during the V pass; the attention scores read q_sb only later)
        for dt in range(HL):
            qsl = q_sb[:, dt * S + (SC - 1) * 512 : dt * S + SC * 512]
            nc.vector.tensor_mul(qsl, qsl, gk[:, (SC - 1) * 512 : SC * 512])

        # wo re-uses wq's SBUF space; the WAR dep on wq's final read delays
        # the load until the QK pass ends, well before p3 needs it
        wo_sb = wq_sb
        for g in range(4):
            w_chunk(wo_sb, woT, g, 4, nc.scalar)

        # ---- phase 1b: V in natural [s, d] layout (x tiles stationary) ----
        with (
            tc.tile_pool(name="pxv", bufs=4) as pxv,
            tc.tile_pool(name="pb_ps", bufs=1, space="PSUM") as pb_ps,
        ):
            for sh in range(2):
                psv = [
                    pb_ps.tile([128, 512], F32, name=f"psv{i}") for i in range(8)
                ]
                for kc in range(KT):
                    xt2 = pxv.tile([128, 1024], BF16, name="xv")
                    nc.sync.dma_start(
                        xt2[:],
                        xT.ap()[
                            kc * 128 : (kc + 1) * 128,
                            sh * 1024 : (sh + 1) * 1024,
                        ],
                    )
                    for st in range(8):
                        nc.tensor.matmul(
                            psv[st][:],
                            xt2[:, st * 128 : (st + 1) * 128],
                            wv_sb[:, kc * DSH : (kc + 1) * DSH],
                            start=(kc == 0),
                            stop=(kc == KT - 1),
                        )
                for st in range(8):
                    gt = sh * 8 + st  # global s-tile 0..15
                    nc.vector.tensor_copy(
                        v_sb[:, gt * DSH : (gt + 1) * DSH], psv[st][:]
                    )

        # ------ phase 2+3: attention, AllGather, output projection ------
        with (
            tc.tile_pool(name="p2_ex", bufs=8) as p2_ex,
            tc.tile_pool(name="p2_sm", bufs=3) as p2_sm,
            tc.tile_pool(name="p2_at", bufs=3) as p2_at,
            tc.tile_pool(name="p2_xs", bufs=2) as p2_xs,
            tc.tile_pool(name="p3_a", bufs=4) as p3_a,
            tc.tile_pool(name="p3_b", bufs=4) as p3_b,
            tc.tile_pool(name="p3_ev", bufs=4) as p3_ev,
        ):
            inv_sqrt_hd = 1.0 / math.sqrt(HD)
            last_at = {}

            def attn_gen(qc, pool_sc, pool_ap, pool_sp):
                """Attention for one 512-q chunk; yields every few score
                tiles so other work can interleave into the issue stream.
                Fires this chunk's AllGather at the end.

                Per-head softmax tail: one [128,128]-ones matmul produces
                the k-sums already broadcast across all partitions (no
                gpsimd broadcast, no DVE-queue blocking), then reciprocal
                and the normalize multiply."""
                live = [kt for kt in range(ST) if cls_grid[kt][qc] != B_SKIP]
                for h in range(HL):
                    qt = q_sb[:, h * S + qc * 512 : h * S + (qc + 1) * 512]
                    att_ps = pool_ap.tile([128, 512], F32, name="att_ps")
                    exs = p2_xs.tile([128, 512], BF16, name="exs")
                    for i, kt in enumerate(live):
                        first = i == 0
                        last = i == len(live) - 1
                        sc_ps = pool_sc.tile([128, 512], F32, name="sc_ps")
                        nc.tensor.matmul(
                            sc_ps[:],
                            k_sb[:, h * S + kt * 128 : h * S + (kt + 1) * 128],
                            qt,
                            start=True,
                            stop=True,
                        )
                        ex = p2_ex.tile([128, 512], BF16, name="ex")
                        last_at["exp"] = nc.scalar.activation(
                            ex[:],
                            sc_ps[:],
                            mybir.ActivationFunctionType.Exp,
                            scale=inv_sqrt_hd,
                        )
                        cls = cls_grid[kt][qc]
                        if cls >= 0:  # mixed block: multiply by exp(mask)
                            nc.vector.tensor_mul(
                                ex[:], ex[:], emu[:, cls * 512 : (cls + 1) * 512]
                            )
                        nc.tensor.matmul(
                            att_ps[:],
                            v_sb[
                                :, kt * DSH + h * 128 : kt * DSH + (h + 1) * 128
                            ],
                            ex[:],
                            start=first,
                            stop=last,
                        )
                        if first:
                            nc.vector.tensor_copy(exs[:], ex[:])
                        else:
                            nc.vector.tensor_add(exs[:], exs[:], ex[:])
                        if i % 4 == 3:
                            yield
                    sumb = pool_sp.tile([128, 512], F32, name="sumb")
                    nc.tensor.matmul(
                        sumb[:], ones_t[:], exs[:], start=True, stop=True
                    )
                    rb = p2_sm.tile([128, 512], F32, name="rb")
                    nc.vector.reciprocal_approx_fast(rb[:], sumb[:])
                    at = p2_at.tile([128, 512], BF16, name="at")
                    nc.vector.tensor_mul(at[:], att_ps[:], rb[:])
                    last_at["sync"] = nc.sync.dma_start(
                        attn_sc[qc].ap()[h * 128 : (h + 1) * 128, :], at[:]
                    )
                    yield
                nc.gpsimd.collective_compute(
                    "AllGather",
                    mybir.AluOpType.bypass,
                    ins=[attn_sc[qc].ap()],
                    outs=[attn_full[qc].ap()],
                    replica_groups=[list(range(N_CORES))],
                )

            def p3_at_load(qc, dc2, eng, pool=None):
                at_t = (pool or p3_a).tile([128, 1024], BF16, name="at_t")
                ld = eng.dma_start(
                    at_t[:].rearrange("p (two s) -> p two s", s=512),
                    attn_full[qc]
                    .ap()[dc2 * 256 : (dc2 + 1) * 256, :]
                    .rearrange("(two p) s -> p two s", p=128),
                )
                if "sync" in last_at:
                    add_dep_helper(
                        ld.ins,
                        last_at["sync"].ins,
                        sync=False,
                        reason="p3 loads ordered after attention DMAs",
                    )
                if eng is nc.scalar and "exp" in last_at:
                    # same-queue anchor: only a scalar-engine dep pins the
                    # scalar queue order (keeps bulk loads behind the exps)
                    add_dep_helper(
                        ld.ins,
                        last_at["exp"].ins,
                        sync=False,
                        reason="scalar p3 loads ordered after exps",
                    )
                return at_t

            def p3_gen(qc, pool_ps, nbank=HL):
                """Single-chunk output projection; with nbank=2 it runs as
                two jt-pair passes over at_t (2 PSUM banks) so it can
                coexist with a deeper attention score pool.  Yields every
                dc2 group for fine-grained interleaving."""
                for rep in range(HL // nbank):
                    pso = [
                        pool_ps.tile([128, 512], F32, name=f"pso{i}")
                        for i in range(nbank)
                    ]
                    for dc2 in range(KT // 2):
                        at_t = p3_at_load(qc, dc2, nc.sync)
                        for half in range(2):
                            dc = dc2 * 2 + half
                            for j in range(nbank):
                                jt = rep * nbank + j
                                nc.tensor.matmul(
                                    pso[j][:],
                                    wo_sb[
                                        :,
                                        dc * DSH
                                        + jt * 128 : dc * DSH
                                        + (jt + 1) * 128,
                                    ],
                                    at_t[:, half * 512 : (half + 1) * 512],
                                    start=(dc == 0),
                                    stop=(dc == KT - 1),
                                )
                        yield
                    for j in range(nbank):
                        jt = rep * nbank + j
                        oev = p3_ev.tile([128, 512], F32, name="oev")
                        nc.scalar.copy(oev[:], pso[j][:])
                        eng = nc.sync if jt % 2 == 0 else nc.scalar
                        eng.dma_start(
                            outT.ap()[
                                jt * 128 : (jt + 1) * 128,
                                qc * 512 : (qc + 1) * 512,
                            ],
                            oev[:],
                        )

            def p3_paired(qca, qcb, pool_ps):
                """Output projection for two chunks together: each wo
                stationary tile feeds two consecutive matmuls."""
                pso = [
                    pool_ps.tile([128, 512], F32, name=f"psp{i}")
                    for i in range(2 * HL)
                ]
                for dc2 in range(KT // 2):
                    at_a = p3_at_load(qca, dc2, nc.scalar, pool=p3_b)
                    at_b = p3_at_load(qcb, dc2, nc.scalar, pool=p3_b)
                    for half in range(2):
                        dc = dc2 * 2 + half
                        for jt in range(HL):
                            for c, at_t in ((0, at_a), (1, at_b)):
                                nc.tensor.matmul(
                                    pso[2 * jt + c][:],
                                    wo_sb[
                                        :,
                                        dc * DSH
                                        + jt * 128 : dc * DSH
                                        + (jt + 1) * 128,
                                    ],
                                    at_t[:, half * 512 : (half + 1) * 512],
                                    start=(dc == 0),
                                    stop=(dc == KT - 1),
                                )
                for jt in range(HL):
                    for c, qc in ((0, qca), (1, qcb)):
                        oev = p3_ev.tile([128, 512], F32, name="oev")
                        nc.scalar.copy(oev[:], pso[2 * jt + c][:])
                        eng = nc.sync if c == 0 else nc.scalar
                        eng.dma_start(
                            outT.ap()[
                                jt * 128 : (jt + 1) * 128,
                                qc * 512 : (qc + 1) * 512,
                            ],
                            oev[:],
                        )

            def run(gen):
                for _ in gen:
                    pass

            def interleave(a, b):
                da = db = False
                while not (da and db):
                    if not da:
                        try:
                            next(a)
                        except StopIteration:
                            da = True
                    if not db:
                        try:
                            next(b)
                        except StopIteration:
                            db = True

            with (
                tc.tile_pool(name="p2_sc", bufs=3, space="PSUM") as psc,
                tc.tile_pool(name="p2_ap", bufs=2, space="PSUM") as pap,
                tc.tile_pool(name="p2_sp", bufs=1, space="PSUM") as psp,
                tc.tile_pool(name="p3q_ps", bufs=1, space="PSUM") as p3q,
            ):
                run(attn_gen(0, psc, pap, psp))
                run(attn_gen(1, psc, pap, psp))
                run(attn_gen(2, psc, pap, psp))
                # p3 for qc0 rides inside qc3's ACT-bound window; its
                # AllGather completed two windows ago
                interleave(attn_gen(3, psc, pap, psp), p3_gen(0, p3q, nbank=2))
            with tc.tile_pool(name="p3p_ps", bufs=1, space="PSUM") as ppair:
                p3_paired(1, 2, ppair)
            with tc.tile_pool(name="p3s_ps", bufs=1, space="PSUM") as psolo:
                run(p3_gen(3, psolo, nbank=2))

    nc.compile()
    return nc


def _install_trace_hooks():
    """Install the NTFF profile hook (missing antenv.axon_hooks stub) and
    neutralize the artifact upload so trace=True works in this container."""
    import sys
    import types

    from concourse import bass_utils as _bu

    _bu.upload_artifacts = lambda tmpdir: f"file://{tmpdir}"
    if "antenv.axon_hooks" in sys.modules:
        return
    import antenv

    mod = types.ModuleType("antenv.axon_hooks")
    _h = [None]
    mod.set_axon_ntff_profile_hook = lambda hk: _h.__setitem__(0, hk)
    mod.get_axon_ntff_profile_hook = lambda: _h[0]
    sys.modules["antenv.axon_hooks"] = mod
    antenv.axon_hooks = mod
    from trn_agent_boot.trn_boot import _ntff_profile_via_ctypes

    mod.set_axon_ntff_profile_hook(
        _ntff_profile_via_ctypes("/opt/axon/libaxon_pjrt.so")
    )


_CACHE = {}


def _get_program(cls_grid, n_uniq):
    key = (tuple(map(tuple, cls_grid)), n_uniq)
    if key not in _CACHE:
        _CACHE[key] = _build(cls_grid, n_uniq)
    return _CACHE[key]


def _classify_mask(maskT_np):
    """Classify each [128k, 512q] block of the transposed mask; dedup the
    mixed blocks into unique resident patterns."""
    grid = []
    uniq = []  # list of (bytes, np block)
    for kt in range(ST):
        row = []
        for qc in range(SC):
            blk = maskT_np[kt * 128 : (kt + 1) * 128, qc * 512 : (qc + 1) * 512]
            if np.all(blk < -1e4):
                row.append(B_SKIP)
            elif np.all(blk == 0.0):
                row.append(B_ZERO)
            else:
                key = blk.tobytes()
                for j, (kb, _) in enumerate(uniq):
                    if kb == key:
                        row.append(j)
                        break
                else:
                    uniq.append((key, blk))
                    row.append(len(uniq) - 1)
        grid.append(row)
    return grid, [b for _, b in uniq]


_ONES = np.ones((128, 128), dtype=BF16_NP)

# within-head permutation: even head_dim indices first, then odd
_PERM = np.empty(DSH, dtype=np.int64)
for _hl in range(HL):
    for _j in range(64):
        _PERM[_hl * 128 + _j] = _hl * 128 + 2 * _j
        _PERM[_hl * 128 + 64 + _j] = _hl * 128 + 2 * _j + 1


def kernel(x, start_pos, freqs, mask, wq, wk, wv, wo):
    x = np.asarray(x, dtype=np.float32)
    freqs = np.asarray(freqs, dtype=np.float32)
    mask = np.asarray(mask, dtype=np.float32)
    wq = np.asarray(wq, dtype=np.float32)
    wk = np.asarray(wk, dtype=np.float32)
    wv = np.asarray(wv, dtype=np.float32)
    wo = np.asarray(wo, dtype=np.float32)

    xs = x.reshape(S, D)
    xT = np.ascontiguousarray(xs.T).astype(BF16_NP)
    fc = freqs[:, :, 0].T  # [64, S] cos_j(s)
    fs = freqs[:, :, 1].T  # [64, S] sin_j(s)
    gkT = np.ascontiguousarray(
        np.concatenate([fc - fs, fc + fs], axis=0).astype(np.float32)
    )
    maskT_np = np.ascontiguousarray(mask.reshape(S, S).T)
    cls_grid, uniq = _classify_mask(maskT_np)
    n_uniq = len(uniq)
    if n_uniq > MAX_UNIQ:
        raise NotImplementedError(f"too many unique mask blocks: {n_uniq}")
    nu = max(n_uniq, 1)
    # multiplicative masking: exp applies scale to scores, so the resident
    # tiles hold exp(mask * scale)
    emu_np = np.zeros((128, nu * 512), dtype=BF16_NP)
    inv = 1.0 / math.sqrt(HD)
    for j, blk in enumerate(uniq):
        emu_np[:, j * 512 : (j + 1) * 512] = np.exp(
            blk.astype(np.float64) * inv
        ).astype(BF16_NP)
    nc = _get_program(cls_grid, n_uniq)

    in_maps = []
    for c in range(N_CORES):
        rows = slice(c * DSH, (c + 1) * DSH)
        wq_c = wq[rows][_PERM]  # permute within-head rows (even hd, odd hd)
        wk_c = wk[rows][_PERM]
        in_maps.append(
            {
                "xT": xT,
                "wqT": np.ascontiguousarray(wq_c.T).astype(BF16_NP),
                "wkT": np.ascontiguousarray(wk_c.T).astype(BF16_NP),
                "wvT": np.ascontiguousarray(wv[rows].T).astype(BF16_NP),
                "woT": np.ascontiguousarray(wo[rows].T).astype(BF16_NP),
                "gkT": gkT,
                "ones_in": _ONES,
                "emu_in": emu_np,
            }
        )

    trace = os.environ.get("ATTN_TRACE") == "1"
    if trace:
        try:
            _install_trace_hooks()
        except Exception:
            pass

    res = run_bass_kernel_spmd(
        nc,
        in_maps,
        list(range(N_CORES)),
        trace=trace,
        trace_cores=[0] if trace else None,
    )
    if trace:
        kernel.last_exec_time_ns = res.exec_time_ns
        kernel.last_results = res

    out = np.empty((S, D), dtype=np.float32)
    for c in range(N_CORES):
        out[:, c * DSH : (c + 1) * DSH] = res.results[c]["outT"].T
    return out[None]
